# revision 1
# baseline (speedup 1.0000x reference)
"""MultiHeadDualAttention Trainium2 kernel, v5 (fp8 DoubleRow + split exp).

Sharding: 8 heads -> 8 cores. Each core: full k1/k2/v1/v2 (host-transposed to
[256, 4096] bf16) + its head's wk/wv slices. Outputs per core: unnormalized
o1T/o2T [65, 4096] bf16 (row 64 = softmax denominator); host divides, applies
the wo projection (row-shard of wo = per-head slice, concat over heads), and
adds the v-bias/wo-bias constants (v-bias commutes through softmax).

Math per head: S[n, m] = k1F[n]·k2F[m] with k1F/k2F the *biased* projections;
rowsoftmax(S) / colsoftmax(S) are exactly the reference's two directions.
Each direction builds its E matrix in the contract-on-partitions layout:
  o2: E[m, n] = exp(SCALE*k2F[m]·k1F[n]), contract over m
  o1: E[n, m] = exp(SCALE*k1F[n]·k2F[m]), contract over n

Perf structure (fp8e4 hot loop):
  - k projections staged fp8 into a DoubleRow layout [32p, 2kt, 4096]
    (d = 2i+t), duplicated at all four partition quarters: the four score
    matmuls of a quad run on PE quarter-tiles (rows 0/32/64/96) as
    concurrent streams.
  - exp: split ACT (exact Exp -> fp8) / DVE (Schraudolph: rn(S*a+b) -> int8
    bitcast fp8e4), strictly alternating; fp8 quantization dominates the
    error of both paths. Measured end relerr ~1.3e-2 (gate 2e-2).
  - PV: fp8 DoubleRow, one matmul per m-tile pair ([128, 2, 512] E x
    [128, 2, 80] v-aug; row pitch 80 because dual-fp8 ldweights needs
    16-divisible strides), accumulating [80, 512] over 16 pairs; row 64
    (ones column) is the softmax denominator.
  - HAM (PE clock-gate at K=4/8 = 1.2 GHz unless kept busy): the PE must be
    the 100%-duty bottleneck engine or the gate re-throttles and never
    recovers (a dependency-stalled stream never reads "busy"). Hence: PE
    work per quad (~1.4us) deliberately exceeds the exp cadence (~1.17us),
    a dependency-free warm burst behind a scheduler fence flips the gate
    right before the attention, and extra burst matmuls are woven between
    the first quads to cover the pipeline-fill stalls.
  - wo projection + normalization on host.
"""

import sys

sys.path.insert(0, "/opt/trn_rl_repo")

import numpy as np

N = 4096
C = 256
D = 64
SCALE = float(D) ** -0.5
NCORES = 8
NCH = 8          # n-chunks of 512
PAIRS = 16       # m-tile pairs (32 m-tiles)
A_SCH = float(8.0 * np.log2(np.e) * SCALE)   # schraudolph multiplier
B_SCH = 55.8                                  # schraudolph magic bias

_cache: dict = {}


def _build_module():
    import concourse.bacc as bacc
    import concourse.mybir as mybir
    import concourse.tile as tile

    f32 = mybir.dt.float32
    bf16 = mybir.dt.bfloat16
    fp8 = mybir.dt.float8e4
    i8 = mybir.dt.int8
    Exp = mybir.ActivationFunctionType.Exp
    Ident = mybir.ActivationFunctionType.Identity
    DR = mybir.MatmulPerfMode.DoubleRow
    Alu = mybir.AluOpType

    nc = bacc.Bacc("TRN2", target_bir_lowering=False, debug=False)

    def din(name, shape, dt=bf16):
        return nc.dram_tensor(name, shape, dt, kind="ExternalInput").ap()

    def dout(name, shape, dt):
        return nc.dram_tensor(name, shape, dt, kind="ExternalOutput").ap()

    k1T = din("k1T", [C, N])
    v1T = din("v1T", [C, N])
    k2T = din("k2T", [C, N])
    v2T = din("v2T", [C, N])
    wk1 = din("wk1", [C, D])
    wk2 = din("wk2", [C, D])
    wv1 = din("wv1", [C, D])
    wv2 = din("wv2", [C, D])
    bk1 = din("bk1", [D, 1], f32)
    bk2 = din("bk2", [D, 1], f32)

    o1Td = dout("o1T", [D + 1, N], bf16)
    o2Td = dout("o2T", [D + 1, N], bf16)

    exp_idx = [0]

    with tile.TileContext(nc) as tc:
        with (
            tc.tile_pool(name="const", bufs=1) as constp,
            tc.tile_pool(name="stg", bufs=4) as stgp,
            tc.tile_pool(name="eblk", bufs=10) as ep,
            tc.tile_pool(name="outp", bufs=4) as outp,
            tc.tile_pool(name="spsum", bufs=3, space="PSUM") as spsum,
            tc.tile_pool(name="opsum", bufs=2, space="PSUM") as opsum,
        ):
            # ---- weights ----
            w_sb = {}
            for name, drt in (("wk1", wk1), ("wk2", wk2), ("wv1", wv1), ("wv2", wv2)):
                t = constp.tile([128, 2, D], bf16, tag=name, name=f"w_{name}")
                for ct in range(2):
                    nc.sync.dma_start(out=t[:, ct, :], in_=drt[ct * 128:(ct + 1) * 128, :])
                w_sb[name] = t
            bk1_sb = constp.tile([D, 1], f32, tag="bk1")
            nc.sync.dma_start(out=bk1_sb[:], in_=bk1[:])
            bk2_sb = constp.tile([D, 1], f32, tag="bk2")
            nc.sync.dma_start(out=bk2_sb[:], in_=bk2[:])

            warm = constp.tile([128, 512], bf16, tag="warm")
            nc.gpsimd.memset(warm[:], 0.0)

            # ---- k projections -> fp8 DoubleRow layout, quarter-duplicated
            # kf[q*32 + i, t, n] = k_proj[d = 2i + t, n] + bias, q in 0..4 ----
            def k_proj_unit(kf, raws, w, b_sb, tag, u):
                stg = stgp.tile([D, 2, 512], fp8, tag="kstg", name=f"kstg_{tag}{u}")
                for jj in range(2):
                    j = 2 * u + jj
                    raw = raws[j]
                    kps = opsum.tile([80, 512], f32, tag="po", name=f"kps_{tag}{j}")
                    for ct in range(2):
                        nc.tensor.matmul(
                            kps[0:D, :], w[:, ct, :], raw[:, ct, :],
                            start=(ct == 0), stop=(ct == 1),
                        )
                    nc.scalar.activation(stg[:, jj, :], kps[0:D, :], Ident, bias=b_sb[:])
                # staging DMAs on the ACT hwdge queue (the SP queue is busy
                # with raw loads; queue-order there would gate the attention)
                for q in range(4):
                    nc.scalar.dma_start(
                        out=kf[q * 32:(q + 1) * 32, :, (2 * u) * 512:(2 * u + 2) * 512],
                        in_=stg[:],
                    )

            # batched raw loads: the head is DMA issue-rate bound (~0.9us of
            # queue time per dma_start), so load 2 chunks per DMA and
            # interleave k1/k2/v2 so v2 lands early (v2aug gates the first
            # PV). The [256, 1024] DRAM block maps to [ct, p, j, n] SBUF
            # order via a rearranged AP (c = ct*128 + p).
            def load_raw(rawT, tg):
                t = constp.tile([128, 2, NCH, 512], bf16, tag=f"{tg}raw",
                                name=f"rawt_{tg}")
                return t

            rawt = {tg: load_raw(rawT, tg)
                    for tg, rawT in (("k1", k1T), ("k2", k2T), ("v2", v2T), ("v1", v1T))}
            for u in range(NCH // 2):
                for tg, rawT in (("k1", k1T), ("k2", k2T), ("v2", v2T)):
                    nc.sync.dma_start(
                        out=rawt[tg][:, :, 2 * u:2 * u + 2, :],
                        in_=rawT[:, (2 * u) * 512:(2 * u + 2) * 512].rearrange(
                            "(c p) (j n) -> p c j n", c=2, j=2),
                    )
            k1raws = [rawt["k1"][:, :, j, :] for j in range(NCH)]
            k2raws = [rawt["k2"][:, :, j, :] for j in range(NCH)]
            v2raws = [rawt["v2"][:, :, j, :] for j in range(NCH)]

            k1f = constp.tile([128, 2, N], fp8, tag="k1f", name="kf_k1f")
            k2f = constp.tile([128, 2, N], fp8, tag="k2f", name="kf_k2f")
            # interleave k1/k2 units so staging issue tracks cast completion
            for u in range(NCH // 2):
                k_proj_unit(k1f, k1raws, w_sb["wk1"], bk1_sb, "k1f", u)
                k_proj_unit(k2f, k2raws, w_sb["wk2"], bk2_sb, "k2f", u)

            # ---- v projections -> fp8 v-aug [128, pair, kt, 80] ----
            # (col 64 = ones for the softmax denominator, 65-79 zero pad)
            def v_proj_compute(raws, w, tag):
                vaug = constp.tile([128, 16, 2, 80], fp8, tag=tag, name=f"vaug_{tag}")
                nc.vector.memset(vaug[:, :, :, D:80], 0.0)
                nc.vector.memset(vaug[:, :, :, D:D + 1], 1.0)
                for half in range(2):
                    vps = spsum.tile([128, 2, 512], f32, tag="sAB", name=f"vps_{tag}{half}")
                    for jj in range(4):
                        raw = raws[half * 4 + jj]
                        for k in range(4):
                            nt_loc = jj * 4 + k
                            out = vps[:, nt_loc // 8, (nt_loc % 8) * D:(nt_loc % 8 + 1) * D]
                            for ct in range(2):
                                nc.tensor.matmul(
                                    out, raw[:, ct, k * 128:(k + 1) * 128], w[:, ct, :],
                                    start=(ct == 0), stop=(ct == 1),
                                )
                    nc.vector.tensor_copy(
                        vaug[:, half * 8:(half + 1) * 8, :, 0:D], vps[:, :, :])
                return vaug

            # ---- attention ----
            def emit_scores_exps(kP, kF, j, g, tag, n_fill):
                """Scores for quad g of chunk j -> two fp8 eblk tiles."""
                ps = [spsum.tile([128, 2, 512], f32, tag="sAB",
                                 name=f"s_{tag}{j}_{g}_{h}")
                      for h in range(2)]
                # HAM filler: dummy matmuls into the quad's own score psum
                # (start=True scores overwrite them; costs no extra banks).
                # Covers the recurring sub-us PE micro-gaps -- without it the
                # clock-gate re-throttles within ~10us of attention start.
                for _ in range(n_fill):
                    nc.tensor.matmul(ps[0][:, 0, 0:256], warm[:, 0:128],
                                     warm[:, 0:256], start=True, stop=True)
                for q in range(4):
                    mt = 4 * g + q
                    nc.tensor.matmul(
                        ps[q // 2][:, q % 2, :],
                        kP[q * 32:(q + 1) * 32, :, mt * 128:(mt + 1) * 128],
                        kF[q * 32:(q + 1) * 32, :, j * 512:(j + 1) * 512],
                        start=True, stop=True,
                        perf_mode=DR, tile_position=(q * 32, 0),
                    )
                ebs = []
                for h in range(2):
                    eblk = ep.tile([128, 2, 512], fp8, tag="eblk",
                                   name=f"e_{tag}{j}_{g}_{h}")
                    idx = exp_idx[0]
                    exp_idx[0] += 1
                    if idx % 2 == 0:
                        nc.scalar.activation(eblk[:, :, :], ps[h][:, :, :],
                                             Exp, scale=SCALE)
                    else:
                        nc.vector.tensor_scalar(
                            eblk[:, :, :].bitcast(i8), ps[h][:, :, :],
                            A_SCH, B_SCH, Alu.mult, Alu.add)
                    ebs.append(eblk)
                return ebs

            def quad(kP, kF, vaug, po, j, g, tag, n_fill=1, pre=None):
                ebs = pre if pre is not None else emit_scores_exps(
                    kP, kF, j, g, tag, n_fill)
                for h in range(2):
                    k = 2 * g + h
                    nc.tensor.matmul(
                        po[:], vaug[:, k, :, :], ebs[h][:, :, :],
                        start=(k == 0), stop=(k == PAIRS - 1),
                        perf_mode=DR,
                    )

            # ---- scheduler fence + HAM warm burst ----
            # Everything DMA-bound stays before the fence; the dependency-free
            # burst after it cannot be hoisted into the idle phase, runs
            # back-to-back, and flips the PE clock-gate to 8/8 right before
            # the attention stream begins.
            tc.no_sync_barrier()

            def warm_burst(n, name):
                wp = spsum.tile([128, 2, 512], f32, tag="sAB", name=name)
                nc.tensor.matmul(
                    wp[:, 0, :], k2f[0:32, :, 4 * 512:4 * 512 + 128],
                    k2f[0:32, :, 4 * 512:5 * 512],
                    start=True, stop=True, perf_mode=DR, tile_position=(0, 0))
                for _ in range(n - 1):
                    nc.tensor.matmul(wp[:, 0, :], warm[:, 0:128], warm[:],
                                     start=True, stop=True)

            warm_burst(26, "warm_att")

            # v1 raw loads after the burst: they stream during early o2 and
            # keep 2MB of DMA out of the pre-attention window (the head is
            # gated by the k-proj matmuls tracking the raw-DMA tail). The
            # projection compute runs at o2 chunk 2, well after they land.
            for u in range(NCH // 2):
                nc.sync.dma_start(
                    out=rawt["v1"][:, :, 2 * u:2 * u + 2, :],
                    in_=v1T[:, (2 * u) * 512:(2 * u + 2) * 512].rearrange(
                        "(c p) (j n) -> p c j n", c=2, j=2),
                )
            v1raws = [rawt["v1"][:, :, j, :] for j in range(NCH)]

            # ---- preheat (emitted before v2aug compute so its score psum
            # tiles are allocated from the pool first and it can run as soon
            # as the early kf staging units land) ----
            preheated = [emit_scores_exps(k2f, k1f, 0, g, "o2", 1)
                         for g in range(3)]

            v2aug = v_proj_compute(v2raws, w_sb["wv2"], "v2aug")

            def run_dir(kP, kF, vaug, oTdr, tag, preheated=(), hook=None):
                for j in range(NCH):
                    po = opsum.tile([80, 512], f32, tag="po", name=f"po_{tag}{j}")
                    for g in range(PAIRS // 2):
                        pre = preheated[g] if j == 0 and g < len(preheated) else None
                        quad(kP, kF, vaug, po, j, g, tag, pre=pre)
                    ot = outp.tile([D + 1, 512], bf16, tag="ot", name=f"ot_{tag}{j}")
                    nc.scalar.copy(ot[:], po[0:D + 1, :])
                    nc.sync.dma_start(out=oTdr[:, j * 512:(j + 1) * 512], in_=ot[:])
                    if hook is not None:
                        hook(j)

            v1aug_h = {}

            def o2_hook(j):
                if j == 2:
                    v1aug_h["t"] = v_proj_compute(v1raws, w_sb["wv1"], "v1aug")

            # o2: E[m, n] = exp(SCALE * k2F[m]*k1F[n]); den over m
            run_dir(k2f, k1f, v2aug, o2Td, "o2", preheated=preheated, hook=o2_hook)
            # o1: E[n, m] = exp(SCALE * k1F[n]*k2F[m]); den over n (o1 scores
            # only need kf, so they fill o2's exp/PV drain -- no gap)
            run_dir(k1f, k2f, v1aug_h["t"], o1Td, "o1")

    nc.compile()
    return nc


def _get_nc():
    if "nc" not in _cache:
        _cache["nc"] = _build_module()
    return _cache["nc"]


def kernel(k1, v1, k2, v2,
           wk1_w, wk1_b, wv1_w, wv1_b,
           wk2_w, wk2_b, wv2_w, wv2_b,
           wo1_w, wo1_b, wo2_w, wo2_b):
    import ml_dtypes
    from concourse.bass_utils import run_bass_kernel_spmd

    nc = _get_nc()

    f = np.float32
    bf = ml_dtypes.bfloat16
    k1T = np.ascontiguousarray(np.asarray(k1, f).T).astype(bf)
    v1T = np.ascontiguousarray(np.asarray(v1, f).T).astype(bf)
    k2T = np.ascontiguousarray(np.asarray(k2, f).T).astype(bf)
    v2T = np.ascontiguousarray(np.asarray(v2, f).T).astype(bf)

    in_maps = []
    for h in range(NCORES):
        sl = slice(h * D, (h + 1) * D)
        in_maps.append({
            "k1T": k1T, "v1T": v1T, "k2T": k2T, "v2T": v2T,
            "wk1": np.ascontiguousarray(np.asarray(wk1_w, f)[:, sl]).astype(bf),
            "wv1": np.ascontiguousarray(np.asarray(wv1_w, f)[:, sl]).astype(bf),
            "wk2": np.ascontiguousarray(np.asarray(wk2_w, f)[:, sl]).astype(bf),
            "wv2": np.ascontiguousarray(np.asarray(wv2_w, f)[:, sl]).astype(bf),
            "bk1": np.ascontiguousarray(np.asarray(wk1_b, f)[sl].reshape(D, 1)),
            "bk2": np.ascontiguousarray(np.asarray(wk2_b, f)[sl].reshape(D, 1)),
        })

    res = run_bass_kernel_spmd(nc, in_maps, list(range(NCORES)))
    _cache["last_result"] = res

    o1_all = np.empty((N, 512), f)
    o2_all = np.empty((N, 512), f)
    for h in range(NCORES):
        rh = res.results[h]
        o1t = np.asarray(rh["o1T"], dtype=f)
        o2t = np.asarray(rh["o2T"], dtype=f)
        o1_all[:, h * D:(h + 1) * D] = (o1t[0:D] / o1t[D:D + 1]).T
        o2_all[:, h * D:(h + 1) * D] = (o2t[0:D] / o2t[D:D + 1]).T
    wo1 = np.asarray(wo1_w, f)
    wo2 = np.asarray(wo2_w, f)
    out1 = o1_all @ wo1 + np.asarray(wv1_b, f) @ wo1 + np.asarray(wo1_b, f)
    out2 = o2_all @ wo2 + np.asarray(wv2_b, f) @ wo2 + np.asarray(wo2_b, f)
    return out1, out2



# revision 4
# speedup vs baseline: 1.3241x; 1.3241x over previous
"""MultiHeadDualAttention Trainium2 kernel, v6 (eblk-granular software pipeline).

Sharding: 8 heads -> 8 cores. Each core: full k1/k2/v1/v2 (host-transposed to
[256, 4096] bf16) + its head's wk/wv slices (wk column-duplicated on host so the
k-projection matmul emits both 64-row-group copies in one shot). Outputs per
core: unnormalized o1T/o2T [65, 4096] bf16 (row 64 = softmax denominator);
host divides, applies the wo projection (row-shard of wo = per-head slice,
concat over heads), and adds the v-bias/wo-bias constants (v-bias commutes
through softmax).

Math per head: S[m, n] = k2F[m]·k1F[n] (o2 direction; o1 swaps k1/k2) with
kF the *biased* projections; rowsoftmax / colsoftmax of the shared score
matrix are exactly the reference's two directions.

Perf structure (measured on HW via microbenchmarks):
  - unit of work = eblk: 2 score m-tiles x one 512-wide n-chunk.
    scores: 2 concurrent no-DR fp8 matmuls on 64-row groups (kf stores the
    projection twice, rows 0-63 / 64-127; warm pair issues in ~216ns).
    exp: full-tile [128,2,512] on ACT (exact Exp) or DVE (Schraudolph
    rn(S*a+b) -> int8 bitcast fp8e4), assigned by a greedy load balancer.
    PV: one fp8 DoubleRow full-array matmul (contract 256) accumulating
    [80,512]; row 64 (ones column) = softmax denominator.
  - software pipeline: PV runs LAG=5 eblks behind its scores so the PE never
    waits on an exp; score-psum pool of 3 [128,2,512] tiles makes the
    write-after-read distance 3 eblks (the 8 PSUM banks allow no more).
  - steady cadence is exp-engine-bound (~700-800ns/eblk); the PE (~65% duty)
    absorbs the k1/v1 projections and HAM filler matmuls in its slack.
  - HAM (PE clock-gate): one dependency-free filler matmul per eblk plus a
    warm burst before the stream keeps the 2.4GHz gate engaged.
  - wo projection + normalization on host.
"""

import sys

sys.path.insert(0, "/opt/trn_rl_repo")

import numpy as np

N = 4096
C = 256
D = 64
SCALE = float(D) ** -0.5
NCORES = 8
NCH = 8          # n-chunks of 512
EPC = 16         # eblks per chunk (16 m-pairs)
LAG = 5          # PV lag in eblks
A_SCH = float(8.0 * np.log2(np.e) * SCALE)   # schraudolph multiplier
B_SCH = 55.8                                  # schraudolph magic bias

_cache: dict = {}


def _build_module():
    import concourse.bacc as bacc
    import concourse.mybir as mybir
    import concourse.tile as tile

    f32 = mybir.dt.float32
    bf16 = mybir.dt.bfloat16
    fp8 = mybir.dt.float8e4
    i8 = mybir.dt.int8
    Exp = mybir.ActivationFunctionType.Exp
    Ident = mybir.ActivationFunctionType.Identity
    DR = mybir.MatmulPerfMode.DoubleRow
    Alu = mybir.AluOpType

    nc = bacc.Bacc("TRN2", target_bir_lowering=False, debug=False)

    def din(name, shape, dt=bf16):
        return nc.dram_tensor(name, shape, dt, kind="ExternalInput").ap()

    def dout(name, shape, dt):
        return nc.dram_tensor(name, shape, dt, kind="ExternalOutput").ap()

    k1T = din("k1T", [C, N])
    v1T = din("v1T", [C, N])
    k2T = din("k2T", [C, N])
    v2T = din("v2T", [C, N])
    wk1 = din("wk1", [C, 2 * D])          # column-duplicated on host
    wk2 = din("wk2", [C, 2 * D])
    wv1 = din("wv1", [C, D])
    wv2 = din("wv2", [C, D])
    bk1 = din("bk1", [2 * D, 1], f32)     # row-duplicated on host
    bk2 = din("bk2", [2 * D, 1], f32)

    o1Td = dout("o1T", [D + 1, N], bf16)
    o2Td = dout("o2T", [D + 1, N], bf16)

    # elementwise-engine load balancer (ns estimates from microbench)
    ew = {"act": 0.0, "dve": 0.0}

    def pick_engine(act_cost, dve_cost):
        if ew["act"] + act_cost <= ew["dve"] + dve_cost:
            ew["act"] += act_cost
            return "act"
        ew["dve"] += dve_cost
        return "dve"

    with tile.TileContext(nc) as tc:
        with (
            tc.tile_pool(name="const", bufs=1) as constp,
            tc.tile_pool(name="eblk", bufs=8) as ep,
            tc.tile_pool(name="outp", bufs=3) as outp,
            tc.tile_pool(name="spsum", bufs=3, space="PSUM") as spsum,
            tc.tile_pool(name="opsum", bufs=2, space="PSUM") as opsum,
        ):
            # ---- weights (gpsimd hwdge queue; SP queue is for raw loads) ----
            w_sb = {}
            for name, drt, cols in (("wk1", wk1, 2 * D), ("wk2", wk2, 2 * D),
                                    ("wv1", wv1, D), ("wv2", wv2, D)):
                t = constp.tile([128, 2, cols], bf16, tag=name, name=f"w_{name}")
                for ct in range(2):
                    nc.gpsimd.dma_start(out=t[:, ct, :], in_=drt[ct * 128:(ct + 1) * 128, :])
                w_sb[name] = t
            bk1_sb = constp.tile([2 * D, 1], f32, tag="bk1")
            nc.gpsimd.dma_start(out=bk1_sb[:], in_=bk1[:])
            bk2_sb = constp.tile([2 * D, 1], f32, tag="bk2")
            nc.gpsimd.dma_start(out=bk2_sb[:], in_=bk2[:])

            warm = constp.tile([128, 512], bf16, tag="warm")
            nc.gpsimd.memset(warm[:], 0.0)

            # ---- raw tensors [128, 2, NCH, 512]; c = ct*128 + p ----
            rawt = {tg: constp.tile([128, 2, NCH, 512], bf16, tag=f"{tg}raw",
                                    name=f"rawt_{tg}")
                    for tg in ("k1", "k2", "v2", "v1")}
            rawd = {"k1": k1T, "k2": k2T, "v2": v2T, "v1": v1T}

            def load_unit(tg, u, eng):
                eng.dma_start(
                    out=rawt[tg][:, :, 2 * u:2 * u + 2, :],
                    in_=rawd[tg][:, (2 * u) * 512:(2 * u + 2) * 512].rearrange(
                        "(c p) (j n) -> p c j n", c=2, j=2))

            # critical-path raws on the SP queue: k1 chunk0 first, then
            # k2/v2 interleaved (m-axis is consumed within the first n-chunk)
            load_unit("k1", 0, nc.sync)
            for u in range(NCH // 2):
                load_unit("k2", u, nc.sync)
                load_unit("v2", u, nc.sync)

            # ---- kf staging: [128, N] fp8, rows 0-63 / 64-127 identical ----
            kf = {"k1": constp.tile([128, N], fp8, tag="k1f", name="kf_k1"),
                  "k2": constp.tile([128, N], fp8, tag="k2f", name="kf_k2")}

            def k_proj_unit(tg, w, b_sb, u):
                """Project chunks 2u, 2u+1 of tg into kf[tg] (both row copies)."""
                ps = spsum.tile([128, 2, 512], f32, tag="sAB", name=f"kp_{tg}{u}")
                for jj in range(2):
                    j = 2 * u + jj
                    for ct in range(2):
                        nc.tensor.matmul(
                            ps[:, jj, :], w[:, ct, :], rawt[tg][:, ct, j, :],
                            start=(ct == 0), stop=(ct == 1))
                nc.scalar.activation(
                    kf[tg][:, (2 * u) * 512:(2 * u + 2) * 512].rearrange(
                        "p (j n) -> p j n", j=2),
                    ps[:], Ident, bias=b_sb[:])
                ew["act"] += 1300.0

            # ---- v projections -> fp8 v-aug [128, pair, kt, 80] ----
            def v_proj_half(raws, w, vaug, half):
                vps = spsum.tile([128, 2, 512], f32, tag="sAB", name=f"vp_{id(vaug)}_{half}")
                for jj in range(4):
                    raw = raws[half * 4 + jj]
                    for k in range(4):
                        nt_loc = jj * 4 + k
                        out = vps[:, nt_loc // 8, (nt_loc % 8) * D:(nt_loc % 8 + 1) * D]
                        for ct in range(2):
                            nc.tensor.matmul(
                                out, raw[:, ct, k * 128:(k + 1) * 128], w[:, ct, :],
                                start=(ct == 0), stop=(ct == 1))
                nc.vector.tensor_copy(
                    vaug[:, half * 8:(half + 1) * 8, :, 0:D], vps[:, :, :])
                ew["dve"] += 1300.0

            def v_aug_alloc(tag):
                vaug = constp.tile([128, 16, 2, 80], fp8, tag=tag, name=f"vaug_{tag}")
                nc.vector.memset(vaug[:, :, :, D:80], 0.0)
                nc.vector.memset(vaug[:, :, :, D:D + 1], 1.0)
                return vaug

            # ---- pre-phase: project k1 unit0, k2 all, v2 all ----
            k_proj_unit("k1", w_sb["wk1"], bk1_sb, 0)
            v2aug = v_aug_alloc("v2aug")
            k_proj_unit("k2", w_sb["wk2"], bk2_sb, 0)
            k_proj_unit("k2", w_sb["wk2"], bk2_sb, 1)
            v2raws = [rawt["v2"][:, :, j, :] for j in range(NCH)]
            v_proj_half(v2raws, w_sb["wv2"], v2aug, 0)
            k_proj_unit("k2", w_sb["wk2"], bk2_sb, 2)
            k_proj_unit("k2", w_sb["wk2"], bk2_sb, 3)
            v_proj_half(v2raws, w_sb["wv2"], v2aug, 1)

            # ---- scheduler fence + HAM warm burst ----
            tc.no_sync_barrier()
            wps = spsum.tile([128, 2, 512], f32, tag="sAB", name="warm_att")
            nc.tensor.matmul(
                wps[:, 0, :], kf["k2"][0:64, 4 * 512:4 * 512 + 128],
                kf["k2"][0:64, 4 * 512:5 * 512], start=True, stop=True,
                tile_position=(0, 0))
            for _ in range(17):
                nc.tensor.matmul(wps[:, 0, :], warm[:, 0:128], warm[:],
                                 start=True, stop=True)

            # ---- attention eblk stream ----
            # directions: o2 (kP=k2f, kF=k1f, v2aug), then o1 (swapped)
            v1aug = v_aug_alloc("v1aug")
            dirs = [("o2", "k2", "k1", v2aug, o2Td), ("o1", "k1", "k2", v1aug, o1Td)]
            NE = 2 * NCH * EPC            # 256 eblks
            pss, ebs = {}, {}
            po_cur = [None]

            def eblk_meta(e):
                d = e // (NCH * EPC)
                r = e % (NCH * EPC)
                return d, r // EPC, r % EPC   # direction, chunk j, pair k

            def s_and_exp(e):
                d, j, k = eblk_meta(e)
                tag, kPn, kFn, vaug, oTd = dirs[d]
                ps = spsum.tile([128, 2, 512], f32, tag="sAB", name=f"ps_{e}")
                pss[e] = ps
                # HAM filler (start=True score overwrites it; no extra banks;
                # 64-row footprint matches the score pair's row class)
                nc.tensor.matmul(ps[:, 0, 0:256], warm[0:64, 0:128],
                                 warm[0:64, 0:256], start=True, stop=True,
                                 tile_position=(0, 0))
                for i in range(2):
                    mt = 2 * k + i
                    h = mt % 2
                    nc.tensor.matmul(
                        ps[:, i, :],
                        kf[kPn][h * D:(h + 1) * D, mt * 128:(mt + 1) * 128],
                        kf[kFn][h * D:(h + 1) * D, j * 512:(j + 1) * 512],
                        start=True, stop=True, tile_position=(h * D, 0))
                eb = ep.tile([128, 2, 512], fp8, tag="eblk", name=f"eb_{e}")
                ebs[e] = eb
                if pick_engine(1150.0, 1260.0) == "act":
                    nc.scalar.activation(eb[:], ps[:], Exp, scale=SCALE)
                else:
                    nc.vector.tensor_scalar(eb[:].bitcast(i8), ps[:],
                                            A_SCH, B_SCH, Alu.mult, Alu.add)
                del pss[e]

            def pv(e):
                d, j, k = eblk_meta(e)
                tag, kPn, kFn, vaug, oTd = dirs[d]
                if k == 0:
                    po_cur[0] = opsum.tile([80, 512], f32, tag="po", name=f"po_{d}{j}")
                nc.tensor.matmul(po_cur[0][:], vaug[:, k, :, :], ebs.pop(e)[:],
                                 start=(k == 0), stop=(k == EPC - 1), perf_mode=DR)
                if k == EPC - 1:
                    ot = outp.tile([D + 1, 512], bf16, tag="ot", name=f"ot_{d}{j}")
                    if pick_engine(810.0, 730.0) == "act":
                        nc.scalar.copy(ot[:], po_cur[0][0:D + 1, :])
                    else:
                        nc.vector.tensor_copy(ot[:], po_cur[0][0:D + 1, :])
                    nc.gpsimd.dma_start(out=oTd[:, j * 512:(j + 1) * 512], in_=ot[:])

            # hooks: woven raw loads + projections during the stream.
            # e is the eblk index at which the work is EMITTED.
            v1raws = [rawt["v1"][:, :, j, :] for j in range(NCH)]
            hooks = {
                1: lambda: load_unit("k1", 1, nc.gpsimd),
                5: lambda: load_unit("k1", 2, nc.gpsimd),
                9: lambda: load_unit("k1", 3, nc.gpsimd),
                13: lambda: load_unit("v1", 0, nc.gpsimd),
                17: lambda: load_unit("v1", 1, nc.gpsimd),
                21: lambda: load_unit("v1", 2, nc.gpsimd),
                25: lambda: load_unit("v1", 3, nc.gpsimd),
                24: lambda: k_proj_unit("k1", w_sb["wk1"], bk1_sb, 1),
                44: lambda: k_proj_unit("k1", w_sb["wk1"], bk1_sb, 2),
                76: lambda: k_proj_unit("k1", w_sb["wk1"], bk1_sb, 3),
                58: lambda: v_proj_half(v1raws, w_sb["wv1"], v1aug, 0),
                90: lambda: v_proj_half(v1raws, w_sb["wv1"], v1aug, 1),
            }

            for e in range(NE + LAG):
                if e >= LAG:
                    pv(e - LAG)
                if e < NE:
                    s_and_exp(e)
                hk = hooks.get(e)
                if hk is not None:
                    hk()

    nc.compile()
    return nc


def _get_nc():
    if "nc" not in _cache:
        _cache["nc"] = _build_module()
    return _cache["nc"]


def kernel(k1, v1, k2, v2,
           wk1_w, wk1_b, wv1_w, wv1_b,
           wk2_w, wk2_b, wv2_w, wv2_b,
           wo1_w, wo1_b, wo2_w, wo2_b):
    import ml_dtypes
    from concourse.bass_utils import run_bass_kernel_spmd

    nc = _get_nc()

    f = np.float32
    bf = ml_dtypes.bfloat16
    k1T = np.ascontiguousarray(np.asarray(k1, f).T).astype(bf)
    v1T = np.ascontiguousarray(np.asarray(v1, f).T).astype(bf)
    k2T = np.ascontiguousarray(np.asarray(k2, f).T).astype(bf)
    v2T = np.ascontiguousarray(np.asarray(v2, f).T).astype(bf)

    in_maps = []
    for h in range(NCORES):
        sl = slice(h * D, (h + 1) * D)

        def dup_w(w):
            ws = np.asarray(w, f)[:, sl]
            return np.ascontiguousarray(np.concatenate([ws, ws], axis=1)).astype(bf)

        def dup_b(b):
            bs = np.asarray(b, f)[sl]
            return np.ascontiguousarray(
                np.concatenate([bs, bs]).reshape(2 * D, 1)).astype(f)

        in_maps.append({
            "k1T": k1T, "v1T": v1T, "k2T": k2T, "v2T": v2T,
            "wk1": dup_w(wk1_w), "wk2": dup_w(wk2_w),
            "wv1": np.ascontiguousarray(np.asarray(wv1_w, f)[:, sl]).astype(bf),
            "wv2": np.ascontiguousarray(np.asarray(wv2_w, f)[:, sl]).astype(bf),
            "bk1": dup_b(wk1_b), "bk2": dup_b(wk2_b),
        })

    res = run_bass_kernel_spmd(nc, in_maps, list(range(NCORES)))
    _cache["last_result"] = res

    o1_all = np.empty((N, 512), f)
    o2_all = np.empty((N, 512), f)
    for h in range(NCORES):
        rh = res.results[h]
        o1t = np.asarray(rh["o1T"], dtype=f)
        o2t = np.asarray(rh["o2T"], dtype=f)
        o1_all[:, h * D:(h + 1) * D] = (o1t[0:D] / o1t[D:D + 1]).T
        o2_all[:, h * D:(h + 1) * D] = (o2t[0:D] / o2t[D:D + 1]).T
    wo1 = np.asarray(wo1_w, f)
    wo2 = np.asarray(wo2_w, f)
    out1 = o1_all @ wo1 + np.asarray(wv1_b, f) @ wo1 + np.asarray(wo1_b, f)
    out2 = o2_all @ wo2 + np.asarray(wv2_b, f) @ wo2 + np.asarray(wo2_b, f)
    return out1, out2


# revision 9
# speedup vs baseline: 1.3447x; 1.0156x over previous
"""MultiHeadDualAttention Trainium2 kernel, v6 (eblk-granular software pipeline).

Sharding: 8 heads -> 8 cores. Each core: full k1/k2/v1/v2 (host-transposed to
[256, 4096] bf16) + its head's wk/wv slices (wk column-duplicated on host so the
k-projection matmul emits both 64-row-group copies in one shot). Outputs per
core: unnormalized o1T/o2T [65, 4096] bf16 (row 64 = softmax denominator);
host divides, applies the wo projection (row-shard of wo = per-head slice,
concat over heads), and adds the v-bias/wo-bias constants (v-bias commutes
through softmax).

Math per head: S[m, n] = k2F[m]·k1F[n] (o2 direction; o1 swaps k1/k2) with
kF the *biased* projections; rowsoftmax / colsoftmax of the shared score
matrix are exactly the reference's two directions.

Perf structure (measured on HW via microbenchmarks):
  - unit of work = eblk: 2 score m-tiles x one 512-wide n-chunk.
    scores: 2 concurrent no-DR fp8 matmuls on 64-row groups (kf stores the
    projection twice, rows 0-63 / 64-127; warm pair issues in ~216ns).
    exp: full-tile [128,2,512] on ACT (exact Exp) or DVE (Schraudolph
    rn(S*a+b) -> int8 bitcast fp8e4), assigned by a greedy load balancer.
    PV: one fp8 DoubleRow full-array matmul (contract 256) accumulating
    [80,512]; row 64 (ones column) = softmax denominator.
  - software pipeline: PV runs LAG=5 eblks behind its scores so the PE never
    waits on an exp; score-psum pool of 3 [128,2,512] tiles makes the
    write-after-read distance 3 eblks (the 8 PSUM banks allow no more).
  - steady cadence is exp-engine-bound (~700-800ns/eblk); the PE (~65% duty)
    absorbs the k1/v1 projections and HAM filler matmuls in its slack.
  - HAM (PE clock-gate): one dependency-free filler matmul per eblk plus a
    warm burst before the stream keeps the 2.4GHz gate engaged.
  - wo projection + normalization on host.
"""

import sys

sys.path.insert(0, "/opt/trn_rl_repo")

import numpy as np

N = 4096
C = 256
D = 64
SCALE = float(D) ** -0.5
NCORES = 8
NCH = 8          # n-chunks of 512
EPC = 16         # eblks per chunk (16 m-pairs)
LAG = 5          # PV lag in eblks
A_SCH = float(8.0 * np.log2(np.e) * SCALE)   # schraudolph multiplier
B_SCH = 55.8                                  # schraudolph magic bias

_cache: dict = {}


def _build_module():
    import concourse.bacc as bacc
    import concourse.mybir as mybir
    import concourse.tile as tile

    f32 = mybir.dt.float32
    bf16 = mybir.dt.bfloat16
    fp8 = mybir.dt.float8e4
    i8 = mybir.dt.int8
    Exp = mybir.ActivationFunctionType.Exp
    Ident = mybir.ActivationFunctionType.Identity
    DR = mybir.MatmulPerfMode.DoubleRow
    Alu = mybir.AluOpType

    nc = bacc.Bacc("TRN2", target_bir_lowering=False, debug=False)

    def din(name, shape, dt=bf16):
        return nc.dram_tensor(name, shape, dt, kind="ExternalInput").ap()

    def dout(name, shape, dt):
        return nc.dram_tensor(name, shape, dt, kind="ExternalOutput").ap()

    k1T = din("k1T", [C, N])
    v1T = din("v1T", [C, N])
    k2T = din("k2T", [C, N])
    v2T = din("v2T", [C, N])
    wk1 = din("wk1", [C, 2 * D])          # column-duplicated on host
    wk2 = din("wk2", [C, 2 * D])
    wv1 = din("wv1", [C, D])
    wv2 = din("wv2", [C, D])
    bk1 = din("bk1", [2 * D, 1], f32)     # row-duplicated on host
    bk2 = din("bk2", [2 * D, 1], f32)

    o1Td = dout("o1T", [D + 1, N], bf16)
    o2Td = dout("o2T", [D + 1, N], bf16)

    # elementwise-engine load balancer (ns estimates from microbench)
    ew = {"act": 0.0, "dve": 0.0}

    def pick_engine(act_cost, dve_cost):
        if ew["act"] + act_cost <= ew["dve"] + dve_cost:
            ew["act"] += act_cost
            return "act"
        ew["dve"] += dve_cost
        return "dve"

    with tile.TileContext(nc) as tc:
        with (
            tc.tile_pool(name="const", bufs=1) as constp,
            tc.tile_pool(name="eblk", bufs=8) as ep,
            tc.tile_pool(name="outp", bufs=3) as outp,
            tc.tile_pool(name="spsum", bufs=3, space="PSUM") as spsum,
            tc.tile_pool(name="opsum", bufs=2, space="PSUM") as opsum,
        ):
            # ---- weights (gpsimd hwdge queue; SP queue is for raw loads) ----
            w_sb = {}
            for name, drt, cols in (("wk1", wk1, 2 * D), ("wk2", wk2, 2 * D),
                                    ("wv1", wv1, D), ("wv2", wv2, D)):
                t = constp.tile([128, 2, cols], bf16, tag=name, name=f"w_{name}")
                for ct in range(2):
                    nc.gpsimd.dma_start(out=t[:, ct, :], in_=drt[ct * 128:(ct + 1) * 128, :])
                w_sb[name] = t
            bk1_sb = constp.tile([2 * D, 1], f32, tag="bk1")
            nc.gpsimd.dma_start(out=bk1_sb[:], in_=bk1[:])
            bk2_sb = constp.tile([2 * D, 1], f32, tag="bk2")
            nc.gpsimd.dma_start(out=bk2_sb[:], in_=bk2[:])

            warm = constp.tile([128, 512], bf16, tag="warm")
            nc.gpsimd.memset(warm[:], 0.0)

            # ---- raw tensors [128, 2, NCH, 512]; c = ct*128 + p ----
            rawt = {tg: constp.tile([128, 2, NCH, 512], bf16, tag=f"{tg}raw",
                                    name=f"rawt_{tg}")
                    for tg in ("k1", "k2", "v2", "v1")}
            rawd = {"k1": k1T, "k2": k2T, "v2": v2T, "v1": v1T}

            def load_unit(tg, u, eng):
                eng.dma_start(
                    out=rawt[tg][:, :, 2 * u:2 * u + 2, :],
                    in_=rawd[tg][:, (2 * u) * 512:(2 * u + 2) * 512].rearrange(
                        "(c p) (j n) -> p c j n", c=2, j=2))

            # raw loads split across TWO hwdge queues (SP + DVE) — one queue
            # only sustains ~half the per-core HBM bandwidth. Order matches
            # consumption: the m-axis (k2/v2) is consumed within the first
            # n-chunk, k1/v1 units later.
            load_unit("k1", 0, nc.sync)
            load_unit("v2", 0, nc.gpsimd)
            load_unit("k2", 0, nc.sync)
            load_unit("k2", 1, nc.gpsimd)
            load_unit("k2", 2, nc.sync)
            load_unit("k2", 3, nc.gpsimd)
            load_unit("v2", 1, nc.sync)
            load_unit("v2", 2, nc.gpsimd)
            load_unit("v2", 3, nc.sync)

            # ---- kf staging: [128, N] fp8, rows 0-63 / 64-127 identical ----
            kf = {"k1": constp.tile([128, N], fp8, tag="k1f", name="kf_k1"),
                  "k2": constp.tile([128, N], fp8, tag="k2f", name="kf_k2")}

            def k_proj_unit(tg, w, b_sb, u):
                """Project chunks 2u, 2u+1 of tg into kf[tg] (both row copies)."""
                ps = spsum.tile([128, 2, 512], f32, tag="sAB", name=f"kp_{tg}{u}")
                for jj in range(2):
                    j = 2 * u + jj
                    for ct in range(2):
                        nc.tensor.matmul(
                            ps[:, jj, :], w[:, ct, :], rawt[tg][:, ct, j, :],
                            start=(ct == 0), stop=(ct == 1))
                nc.scalar.activation(
                    kf[tg][:, (2 * u) * 512:(2 * u + 2) * 512].rearrange(
                        "p (j n) -> p j n", j=2),
                    ps[:], Ident, bias=b_sb[:])
                ew["act"] += 1300.0

            # ---- v projections -> fp8 v-aug [128, pair, kt, 80] ----
            # per-unit granularity (2 chunks -> pairs 4u..4u+3) so v-aug
            # tracks the raw-load pipeline
            def v_proj_unit(raws, w, vaug, u):
                vps = spsum.tile([128, 2, 512], f32, tag="sAB",
                                 name=f"vp_{id(vaug)}_{u}")
                for jj in range(2):
                    raw = raws[2 * u + jj]
                    for k in range(4):
                        nt_loc = jj * 4 + k
                        out = vps[:, 0, nt_loc * D:(nt_loc + 1) * D]
                        for ct in range(2):
                            nc.tensor.matmul(
                                out, raw[:, ct, k * 128:(k + 1) * 128], w[:, ct, :],
                                start=(ct == 0), stop=(ct == 1))
                nc.vector.tensor_copy(
                    vaug[:, 4 * u:4 * (u + 1), :, 0:D], vps[:, 0, :])
                ew["dve"] += 730.0

            def v_aug_alloc(tag):
                vaug = constp.tile([128, 16, 2, 80], fp8, tag=tag, name=f"vaug_{tag}")
                nc.vector.memset(vaug[:, :, :, D:80], 0.0)
                nc.vector.memset(vaug[:, :, :, D:D + 1], 1.0)
                return vaug

            # ---- pre-phase: only what eblk 0-7 needs ----
            v2aug = v_aug_alloc("v2aug")
            v2raws = [rawt["v2"][:, :, j, :] for j in range(NCH)]
            k_proj_unit("k1", w_sb["wk1"], bk1_sb, 0)
            k_proj_unit("k2", w_sb["wk2"], bk2_sb, 0)
            v_proj_unit(v2raws, w_sb["wv2"], v2aug, 0)
            k_proj_unit("k2", w_sb["wk2"], bk2_sb, 1)

            # ---- scheduler fence + HAM warm burst ----
            tc.no_sync_barrier()
            wps = spsum.tile([128, 2, 512], f32, tag="sAB", name="warm_att")
            nc.tensor.matmul(
                wps[:, 0, :], kf["k2"][0:64, 512:512 + 128],
                kf["k2"][0:64, 0:512], start=True, stop=True,
                tile_position=(0, 0))
            for _ in range(13):
                nc.tensor.matmul(wps[:, 0, :], warm[:, 0:128], warm[:],
                                 start=True, stop=True)

            # ---- attention eblk stream ----
            # directions: o2 (kP=k2f, kF=k1f, v2aug), then o1 (swapped)
            v1aug = v_aug_alloc("v1aug")
            dirs = [("o2", "k2", "k1", v2aug, o2Td), ("o1", "k1", "k2", v1aug, o1Td)]
            NE = 2 * NCH * EPC            # 256 eblks
            pss, ebs = {}, {}
            po_cur = [None]

            def eblk_meta(e):
                d = e // (NCH * EPC)
                r = e % (NCH * EPC)
                return d, r // EPC, r % EPC   # direction, chunk j, pair k

            def s_and_exp(e):
                d, j, k = eblk_meta(e)
                tag, kPn, kFn, vaug, oTd = dirs[d]
                ps = spsum.tile([128, 2, 512], f32, tag="sAB", name=f"ps_{e}")
                pss[e] = ps
                # HAM filler (start=True score overwrites it; no extra banks;
                # 64-row footprint matches the score pair's row class)
                nc.tensor.matmul(ps[:, 0, 0:256], warm[0:64, 0:128],
                                 warm[0:64, 0:256], start=True, stop=True,
                                 tile_position=(0, 0))
                for i in range(2):
                    mt = 2 * k + i
                    h = mt % 2
                    nc.tensor.matmul(
                        ps[:, i, :],
                        kf[kPn][h * D:(h + 1) * D, mt * 128:(mt + 1) * 128],
                        kf[kFn][h * D:(h + 1) * D, j * 512:(j + 1) * 512],
                        start=True, stop=True, tile_position=(h * D, 0))
                eb = ep.tile([128, 2, 512], fp8, tag="eblk", name=f"eb_{e}")
                ebs[e] = eb
                if pick_engine(1150.0, 1260.0) == "act":
                    nc.scalar.activation(eb[:], ps[:], Exp, scale=SCALE)
                else:
                    nc.vector.tensor_scalar(eb[:].bitcast(i8), ps[:],
                                            A_SCH, B_SCH, Alu.mult, Alu.add)
                del pss[e]

            def pv(e):
                d, j, k = eblk_meta(e)
                tag, kPn, kFn, vaug, oTd = dirs[d]
                if k == 0:
                    po_cur[0] = opsum.tile([80, 512], f32, tag="po", name=f"po_{d}{j}")
                nc.tensor.matmul(po_cur[0][:], vaug[:, k, :, :], ebs.pop(e)[:],
                                 start=(k == 0), stop=(k == EPC - 1), perf_mode=DR)
                if k == EPC - 1:
                    ot = outp.tile([D + 1, 512], bf16, tag="ot", name=f"ot_{d}{j}")
                    if pick_engine(810.0, 730.0) == "act":
                        nc.scalar.copy(ot[:], po_cur[0][0:D + 1, :])
                    else:
                        nc.vector.tensor_copy(ot[:], po_cur[0][0:D + 1, :])
                    nc.gpsimd.dma_start(out=oTd[:, j * 512:(j + 1) * 512], in_=ot[:])

            # hooks: woven raw loads + projections during the stream.
            # e is the eblk index at which the work is EMITTED.
            v1raws = [rawt["v1"][:, :, j, :] for j in range(NCH)]
            hooks = {
                1: lambda: load_unit("k1", 1, nc.gpsimd),
                2: lambda: k_proj_unit("k2", w_sb["wk2"], bk2_sb, 2),
                3: lambda: load_unit("k1", 2, nc.sync),
                4: lambda: k_proj_unit("k2", w_sb["wk2"], bk2_sb, 3),
                5: lambda: load_unit("k1", 3, nc.gpsimd),
                6: lambda: v_proj_unit(v2raws, w_sb["wv2"], v2aug, 1),
                7: lambda: load_unit("v1", 0, nc.sync),
                8: lambda: v_proj_unit(v2raws, w_sb["wv2"], v2aug, 2),
                9: lambda: load_unit("v1", 1, nc.gpsimd),
                10: lambda: v_proj_unit(v2raws, w_sb["wv2"], v2aug, 3),
                11: lambda: load_unit("v1", 2, nc.sync),
                13: lambda: load_unit("v1", 3, nc.gpsimd),
                22: lambda: k_proj_unit("k1", w_sb["wk1"], bk1_sb, 1),
                40: lambda: k_proj_unit("k1", w_sb["wk1"], bk1_sb, 2),
                56: lambda: k_proj_unit("k1", w_sb["wk1"], bk1_sb, 3),
                72: lambda: v_proj_unit(v1raws, w_sb["wv1"], v1aug, 0),
                80: lambda: v_proj_unit(v1raws, w_sb["wv1"], v1aug, 1),
                88: lambda: v_proj_unit(v1raws, w_sb["wv1"], v1aug, 2),
                96: lambda: v_proj_unit(v1raws, w_sb["wv1"], v1aug, 3),
            }

            for e in range(NE + LAG):
                if e >= LAG:
                    pv(e - LAG)
                if e < NE:
                    s_and_exp(e)
                hk = hooks.get(e)
                if hk is not None:
                    hk()

    nc.compile()
    return nc


def _get_nc():
    if "nc" not in _cache:
        _cache["nc"] = _build_module()
    return _cache["nc"]


def kernel(k1, v1, k2, v2,
           wk1_w, wk1_b, wv1_w, wv1_b,
           wk2_w, wk2_b, wv2_w, wv2_b,
           wo1_w, wo1_b, wo2_w, wo2_b):
    import ml_dtypes
    from concourse.bass_utils import run_bass_kernel_spmd

    nc = _get_nc()

    f = np.float32
    bf = ml_dtypes.bfloat16
    k1T = np.ascontiguousarray(np.asarray(k1, f).T).astype(bf)
    v1T = np.ascontiguousarray(np.asarray(v1, f).T).astype(bf)
    k2T = np.ascontiguousarray(np.asarray(k2, f).T).astype(bf)
    v2T = np.ascontiguousarray(np.asarray(v2, f).T).astype(bf)

    in_maps = []
    for h in range(NCORES):
        sl = slice(h * D, (h + 1) * D)

        def dup_w(w):
            ws = np.asarray(w, f)[:, sl]
            return np.ascontiguousarray(np.concatenate([ws, ws], axis=1)).astype(bf)

        def dup_b(b):
            bs = np.asarray(b, f)[sl]
            return np.ascontiguousarray(
                np.concatenate([bs, bs]).reshape(2 * D, 1)).astype(f)

        in_maps.append({
            "k1T": k1T, "v1T": v1T, "k2T": k2T, "v2T": v2T,
            "wk1": dup_w(wk1_w), "wk2": dup_w(wk2_w),
            "wv1": np.ascontiguousarray(np.asarray(wv1_w, f)[:, sl]).astype(bf),
            "wv2": np.ascontiguousarray(np.asarray(wv2_w, f)[:, sl]).astype(bf),
            "bk1": dup_b(wk1_b), "bk2": dup_b(wk2_b),
        })

    res = run_bass_kernel_spmd(nc, in_maps, list(range(NCORES)))
    _cache["last_result"] = res

    o1_all = np.empty((N, 512), f)
    o2_all = np.empty((N, 512), f)
    for h in range(NCORES):
        rh = res.results[h]
        o1t = np.asarray(rh["o1T"], dtype=f)
        o2t = np.asarray(rh["o2T"], dtype=f)
        o1_all[:, h * D:(h + 1) * D] = (o1t[0:D] / o1t[D:D + 1]).T
        o2_all[:, h * D:(h + 1) * D] = (o2t[0:D] / o2t[D:D + 1]).T
    wo1 = np.asarray(wo1_w, f)
    wo2 = np.asarray(wo2_w, f)
    out1 = o1_all @ wo1 + np.asarray(wv1_b, f) @ wo1 + np.asarray(wo1_b, f)
    out2 = o2_all @ wo2 + np.asarray(wv2_b, f) @ wo2 + np.asarray(wo2_b, f)
    return out1, out2


# revision 11
# speedup vs baseline: 1.3465x; 1.0013x over previous
"""MultiHeadDualAttention Trainium2 kernel, v6 (eblk-granular software pipeline).

Sharding: 8 heads -> 8 cores. Each core: full k1/k2/v1/v2 (host-transposed to
[256, 4096] bf16) + its head's wk/wv slices (wk column-duplicated on host so the
k-projection matmul emits both 64-row-group copies in one shot). Outputs per
core: unnormalized o1T/o2T [65, 4096] bf16 (row 64 = softmax denominator);
host divides, applies the wo projection (row-shard of wo = per-head slice,
concat over heads), and adds the v-bias/wo-bias constants (v-bias commutes
through softmax).

Math per head: S[m, n] = k2F[m]·k1F[n] (o2 direction; o1 swaps k1/k2) with
kF the *biased* projections; rowsoftmax / colsoftmax of the shared score
matrix are exactly the reference's two directions.

Perf structure (measured on HW via microbenchmarks):
  - unit of work = eblk: 2 score m-tiles x one 512-wide n-chunk.
    scores: 2 concurrent no-DR fp8 matmuls on 64-row groups (kf stores the
    projection twice, rows 0-63 / 64-127; warm pair issues in ~216ns).
    exp: full-tile [128,2,512] on ACT (exact Exp) or DVE (Schraudolph
    rn(S*a+b) -> int8 bitcast fp8e4), assigned by a greedy load balancer.
    PV: one fp8 DoubleRow full-array matmul (contract 256) accumulating
    [80,512]; row 64 (ones column) = softmax denominator.
  - software pipeline: PV runs LAG=5 eblks behind its scores so the PE never
    waits on an exp; score-psum pool of 3 [128,2,512] tiles makes the
    write-after-read distance 3 eblks (the 8 PSUM banks allow no more).
  - steady cadence is exp-engine-bound (~700-800ns/eblk); the PE (~65% duty)
    absorbs the k1/v1 projections and HAM filler matmuls in its slack.
  - HAM (PE clock-gate): one dependency-free filler matmul per eblk plus a
    warm burst before the stream keeps the 2.4GHz gate engaged.
  - wo projection + normalization on host.
"""

import sys

sys.path.insert(0, "/opt/trn_rl_repo")

import numpy as np

N = 4096
C = 256
D = 64
SCALE = float(D) ** -0.5
NCORES = 8
NCH = 8          # n-chunks of 512
EPC = 16         # eblks per chunk (16 m-pairs)
LAG = 5          # PV lag in eblks
A_SCH = float(8.0 * np.log2(np.e) * SCALE)   # schraudolph multiplier
B_SCH = 55.8                                  # schraudolph magic bias

_cache: dict = {}


def _build_module():
    import concourse.bacc as bacc
    import concourse.mybir as mybir
    import concourse.tile as tile

    f32 = mybir.dt.float32
    bf16 = mybir.dt.bfloat16
    fp8 = mybir.dt.float8e4
    i8 = mybir.dt.int8
    Exp = mybir.ActivationFunctionType.Exp
    Ident = mybir.ActivationFunctionType.Identity
    DR = mybir.MatmulPerfMode.DoubleRow
    Alu = mybir.AluOpType

    nc = bacc.Bacc("TRN2", target_bir_lowering=False, debug=False)

    def din(name, shape, dt=bf16):
        return nc.dram_tensor(name, shape, dt, kind="ExternalInput").ap()

    def dout(name, shape, dt):
        return nc.dram_tensor(name, shape, dt, kind="ExternalOutput").ap()

    k1T = din("k1T", [C, N])
    v1T = din("v1T", [C, N])
    k2T = din("k2T", [C, N])
    v2T = din("v2T", [C, N])
    wk1 = din("wk1", [C, 2 * D])          # column-duplicated on host
    wk2 = din("wk2", [C, 2 * D])
    wv1 = din("wv1", [C, D])
    wv2 = din("wv2", [C, D])
    bk1 = din("bk1", [2 * D, 1], f32)     # row-duplicated on host
    bk2 = din("bk2", [2 * D, 1], f32)

    o1Td = dout("o1T", [D + 1, N], bf16)
    o2Td = dout("o2T", [D + 1, N], bf16)

    # elementwise-engine load balancer (ns estimates from microbench)
    ew = {"act": 0.0, "dve": 0.0}

    def pick_engine(act_cost, dve_cost):
        if ew["act"] + act_cost <= ew["dve"] + dve_cost:
            ew["act"] += act_cost
            return "act"
        ew["dve"] += dve_cost
        return "dve"

    with tile.TileContext(nc) as tc:
        with (
            tc.tile_pool(name="const", bufs=1) as constp,
            tc.tile_pool(name="eblk", bufs=8) as ep,
            tc.tile_pool(name="outp", bufs=3) as outp,
            tc.tile_pool(name="spsum", bufs=3, space="PSUM") as spsum,
            tc.tile_pool(name="opsum", bufs=2, space="PSUM") as opsum,
        ):
            # ---- weights (gpsimd hwdge queue; SP queue is for raw loads) ----
            w_sb = {}
            for name, drt, cols in (("wk1", wk1, 2 * D), ("wk2", wk2, 2 * D),
                                    ("wv1", wv1, D), ("wv2", wv2, D)):
                t = constp.tile([128, 2, cols], bf16, tag=name, name=f"w_{name}")
                for ct in range(2):
                    nc.gpsimd.dma_start(out=t[:, ct, :], in_=drt[ct * 128:(ct + 1) * 128, :])
                w_sb[name] = t
            bk1_sb = constp.tile([2 * D, 1], f32, tag="bk1")
            nc.gpsimd.dma_start(out=bk1_sb[:], in_=bk1[:])
            bk2_sb = constp.tile([2 * D, 1], f32, tag="bk2")
            nc.gpsimd.dma_start(out=bk2_sb[:], in_=bk2[:])

            warm = constp.tile([128, 512], bf16, tag="warm")
            nc.gpsimd.memset(warm[:], 0.0)

            # ---- raw tensors [128, 2, NCH, 512]; c = ct*128 + p ----
            rawt = {tg: constp.tile([128, 2, NCH, 512], bf16, tag=f"{tg}raw",
                                    name=f"rawt_{tg}")
                    for tg in ("k1", "k2", "v2", "v1")}
            rawd = {"k1": k1T, "k2": k2T, "v2": v2T, "v1": v1T}

            def load_unit(tg, u, eng):
                eng.dma_start(
                    out=rawt[tg][:, :, 2 * u:2 * u + 2, :],
                    in_=rawd[tg][:, (2 * u) * 512:(2 * u + 2) * 512].rearrange(
                        "(c p) (j n) -> p c j n", c=2, j=2))

            # raw loads split across TWO hwdge queues (SP + DVE) — one queue
            # only sustains ~half the per-core HBM bandwidth. Order matches
            # consumption: the m-axis (k2/v2) is consumed within the first
            # n-chunk, k1/v1 units later.
            load_unit("k1", 0, nc.sync)
            load_unit("v2", 0, nc.gpsimd)
            load_unit("k2", 0, nc.sync)
            load_unit("k2", 1, nc.gpsimd)
            load_unit("k2", 2, nc.sync)
            load_unit("k2", 3, nc.gpsimd)
            load_unit("v2", 1, nc.sync)
            load_unit("v2", 2, nc.gpsimd)
            load_unit("v2", 3, nc.sync)

            # ---- kf staging: [128, N] fp8, rows 0-63 / 64-127 identical ----
            kf = {"k1": constp.tile([128, N], fp8, tag="k1f", name="kf_k1"),
                  "k2": constp.tile([128, N], fp8, tag="k2f", name="kf_k2")}

            def k_proj_chunk(tg, w, b_sb, j):
                """Project chunk j of tg into kf[tg] (both row copies). Uses an
                opsum-pool tile so the score-psum pool is never borrowed."""
                ps = opsum.tile([128, 512], f32, tag="po", name=f"kp_{tg}{j}")
                for ct in range(2):
                    nc.tensor.matmul(
                        ps[:], w[:, ct, :], rawt[tg][:, ct, j, :],
                        start=(ct == 0), stop=(ct == 1))
                nc.scalar.activation(
                    kf[tg][:, j * 512:(j + 1) * 512], ps[:], Ident, bias=b_sb[:])
                ew["act"] += 700.0

            # ---- v projections -> fp8 v-aug [128, pair, kt, 80] ----
            # per-unit granularity (2 chunks -> pairs 4u..4u+3) so v-aug
            # tracks the raw-load pipeline
            def v_proj_unit(raws, w, vaug, u):
                vps = opsum.tile([128, 512], f32, tag="po",
                                 name=f"vp_{id(vaug)}_{u}")
                for jj in range(2):
                    raw = raws[2 * u + jj]
                    for k in range(4):
                        nt_loc = jj * 4 + k
                        out = vps[:, nt_loc * D:(nt_loc + 1) * D]
                        for ct in range(2):
                            nc.tensor.matmul(
                                out, raw[:, ct, k * 128:(k + 1) * 128], w[:, ct, :],
                                start=(ct == 0), stop=(ct == 1))
                nc.vector.tensor_copy(
                    vaug[:, 4 * u:4 * (u + 1), :, 0:D], vps[:])
                ew["dve"] += 730.0

            def v_aug_alloc(tag):
                vaug = constp.tile([128, 16, 2, 80], fp8, tag=tag, name=f"vaug_{tag}")
                nc.vector.memset(vaug[:, :, :, D:80], 0.0)
                nc.vector.memset(vaug[:, :, :, D:D + 1], 1.0)
                return vaug

            # ---- HAM warm burst: dependency-free, issues during the raw
            # loads so the clock gate is open when eblk 0 starts ----
            wps = spsum.tile([128, 2, 512], f32, tag="sAB", name="warm_att")
            for _ in range(15):
                nc.tensor.matmul(wps[:, 0, :], warm[:, 0:128], warm[:],
                                 start=True, stop=True)

            # ---- pre-phase: only what eblk 0-3 needs ----
            v2aug = v_aug_alloc("v2aug")
            v2raws = [rawt["v2"][:, :, j, :] for j in range(NCH)]
            k_proj_chunk("k1", w_sb["wk1"], bk1_sb, 0)
            k_proj_chunk("k1", w_sb["wk1"], bk1_sb, 1)
            k_proj_chunk("k2", w_sb["wk2"], bk2_sb, 0)
            k_proj_chunk("k2", w_sb["wk2"], bk2_sb, 1)
            v_proj_unit(v2raws, w_sb["wv2"], v2aug, 0)

            # ---- attention eblk stream ----
            # directions: o2 (kP=k2f, kF=k1f, v2aug), then o1 (swapped)
            v1aug = v_aug_alloc("v1aug")
            dirs = [("o2", "k2", "k1", v2aug, o2Td), ("o1", "k1", "k2", v1aug, o1Td)]
            NE = 2 * NCH * EPC            # 256 eblks
            pss, ebs = {}, {}
            po_cur = [None]

            def eblk_meta(e):
                d = e // (NCH * EPC)
                r = e % (NCH * EPC)
                return d, r // EPC, r % EPC   # direction, chunk j, pair k

            def s_and_exp(e):
                d, j, k = eblk_meta(e)
                tag, kPn, kFn, vaug, oTd = dirs[d]
                ps = spsum.tile([128, 2, 512], f32, tag="sAB", name=f"ps_{e}")
                pss[e] = ps
                # HAM filler (start=True score overwrites it; no extra banks;
                # 64-row footprint matches the score pair's row class)
                nc.tensor.matmul(ps[:, 0, 0:256], warm[0:64, 0:128],
                                 warm[0:64, 0:256], start=True, stop=True,
                                 tile_position=(0, 0))
                for i in range(2):
                    mt = 2 * k + i
                    h = mt % 2
                    nc.tensor.matmul(
                        ps[:, i, :],
                        kf[kPn][h * D:(h + 1) * D, mt * 128:(mt + 1) * 128],
                        kf[kFn][h * D:(h + 1) * D, j * 512:(j + 1) * 512],
                        start=True, stop=True, tile_position=(h * D, 0))
                eb = ep.tile([128, 2, 512], fp8, tag="eblk", name=f"eb_{e}")
                ebs[e] = eb
                if pick_engine(1150.0, 1260.0) == "act":
                    nc.scalar.activation(eb[:], ps[:], Exp, scale=SCALE)
                else:
                    nc.vector.tensor_scalar(eb[:].bitcast(i8), ps[:],
                                            A_SCH, B_SCH, Alu.mult, Alu.add)
                del pss[e]

            def pv(e):
                d, j, k = eblk_meta(e)
                tag, kPn, kFn, vaug, oTd = dirs[d]
                if k == 0:
                    po_cur[0] = opsum.tile([80, 512], f32, tag="po", name=f"po_{d}{j}")
                nc.tensor.matmul(po_cur[0][:], vaug[:, k, :, :], ebs.pop(e)[:],
                                 start=(k == 0), stop=(k == EPC - 1), perf_mode=DR)
                if k == EPC - 1:
                    ot = outp.tile([D + 1, 512], bf16, tag="ot", name=f"ot_{d}{j}")
                    if pick_engine(810.0, 730.0) == "act":
                        nc.scalar.copy(ot[:], po_cur[0][0:D + 1, :])
                    else:
                        nc.vector.tensor_copy(ot[:], po_cur[0][0:D + 1, :])
                    nc.gpsimd.dma_start(out=oTd[:, j * 512:(j + 1) * 512], in_=ot[:])

            # hooks: woven raw loads + projections during the stream.
            # e is the eblk index at which the work is EMITTED.
            v1raws = [rawt["v1"][:, :, j, :] for j in range(NCH)]
            def kpc(tg, j):
                names = {"k1": (w_sb["wk1"], bk1_sb), "k2": (w_sb["wk2"], bk2_sb)}
                w, b = names[tg]
                return lambda: k_proj_chunk(tg, w, b, j)

            hooks = {
                1: [kpc("k2", 2), lambda: load_unit("k1", 1, nc.gpsimd)],
                2: [kpc("k2", 3), lambda: v_proj_unit(v2raws, w_sb["wv2"], v2aug, 1)],
                3: [kpc("k2", 4), lambda: load_unit("k1", 2, nc.sync)],
                5: [kpc("k2", 5), lambda: load_unit("k1", 3, nc.gpsimd)],
                7: [kpc("k2", 6), lambda: v_proj_unit(v2raws, w_sb["wv2"], v2aug, 2)],
                9: [kpc("k2", 7), lambda: load_unit("v1", 0, nc.sync)],
                11: [lambda: v_proj_unit(v2raws, w_sb["wv2"], v2aug, 3),
                     lambda: load_unit("v1", 1, nc.gpsimd)],
                13: [lambda: load_unit("v1", 2, nc.sync)],
                15: [lambda: load_unit("v1", 3, nc.gpsimd)],
                24: [kpc("k1", 2)],
                30: [kpc("k1", 3)],
                46: [kpc("k1", 4)],
                62: [kpc("k1", 5)],
                78: [kpc("k1", 6)],
                94: [kpc("k1", 7)],
                98: [lambda: v_proj_unit(v1raws, w_sb["wv1"], v1aug, 0)],
                106: [lambda: v_proj_unit(v1raws, w_sb["wv1"], v1aug, 1)],
                114: [lambda: v_proj_unit(v1raws, w_sb["wv1"], v1aug, 2)],
                122: [lambda: v_proj_unit(v1raws, w_sb["wv1"], v1aug, 3)],
            }

            tfill = [None]
            for e in range(NE + LAG):
                if e >= LAG:
                    pv(e - LAG)
                if e < NE:
                    s_and_exp(e)
                else:
                    # trailing fills keep the PE clock gate open while the
                    # last exps/PVs drain (fresh tile: wps' buffer was
                    # recycled by the score-psum rotation long ago)
                    if tfill[0] is None:
                        tfill[0] = spsum.tile([128, 2, 512], f32, tag="sAB",
                                              name="tail_fill")
                    nc.tensor.matmul(tfill[0][:, 0, :], warm[:, 0:128], warm[:],
                                     start=True, stop=True)
                for hk in hooks.get(e, ()):
                    hk()

    nc.compile()
    return nc


def _get_nc():
    if "nc" not in _cache:
        _cache["nc"] = _build_module()
    return _cache["nc"]


def kernel(k1, v1, k2, v2,
           wk1_w, wk1_b, wv1_w, wv1_b,
           wk2_w, wk2_b, wv2_w, wv2_b,
           wo1_w, wo1_b, wo2_w, wo2_b):
    import ml_dtypes
    from concourse.bass_utils import run_bass_kernel_spmd

    nc = _get_nc()

    f = np.float32
    bf = ml_dtypes.bfloat16
    k1T = np.ascontiguousarray(np.asarray(k1, f).T).astype(bf)
    v1T = np.ascontiguousarray(np.asarray(v1, f).T).astype(bf)
    k2T = np.ascontiguousarray(np.asarray(k2, f).T).astype(bf)
    v2T = np.ascontiguousarray(np.asarray(v2, f).T).astype(bf)

    in_maps = []
    for h in range(NCORES):
        sl = slice(h * D, (h + 1) * D)

        def dup_w(w):
            ws = np.asarray(w, f)[:, sl]
            return np.ascontiguousarray(np.concatenate([ws, ws], axis=1)).astype(bf)

        def dup_b(b):
            bs = np.asarray(b, f)[sl]
            return np.ascontiguousarray(
                np.concatenate([bs, bs]).reshape(2 * D, 1)).astype(f)

        in_maps.append({
            "k1T": k1T, "v1T": v1T, "k2T": k2T, "v2T": v2T,
            "wk1": dup_w(wk1_w), "wk2": dup_w(wk2_w),
            "wv1": np.ascontiguousarray(np.asarray(wv1_w, f)[:, sl]).astype(bf),
            "wv2": np.ascontiguousarray(np.asarray(wv2_w, f)[:, sl]).astype(bf),
            "bk1": dup_b(wk1_b), "bk2": dup_b(wk2_b),
        })

    res = run_bass_kernel_spmd(nc, in_maps, list(range(NCORES)))
    _cache["last_result"] = res

    o1_all = np.empty((N, 512), f)
    o2_all = np.empty((N, 512), f)
    for h in range(NCORES):
        rh = res.results[h]
        o1t = np.asarray(rh["o1T"], dtype=f)
        o2t = np.asarray(rh["o2T"], dtype=f)
        o1_all[:, h * D:(h + 1) * D] = (o1t[0:D] / o1t[D:D + 1]).T
        o2_all[:, h * D:(h + 1) * D] = (o2t[0:D] / o2t[D:D + 1]).T
    wo1 = np.asarray(wo1_w, f)
    wo2 = np.asarray(wo2_w, f)
    out1 = o1_all @ wo1 + np.asarray(wv1_b, f) @ wo1 + np.asarray(wo1_b, f)
    out2 = o2_all @ wo2 + np.asarray(wv2_b, f) @ wo2 + np.asarray(wo2_b, f)
    return out1, out2
